# revision 5
# baseline (speedup 1.0000x reference)
import math
from functools import partial

import jax
import jax.numpy as jnp
import numpy as np

# nn_AFM_PRBS: data-parallel over batch across 8 NeuronCores.
# B=16, H=W=256, N_BINS=100. Each core gets B/8 = 2 batches; all params
# are replicated (tiny CNN + FC); dist/radius constants are recomputed
# per device (they are static functions of H, W).

_B, _H, _W = 16, 256, 256
_NCORES = 8
_NBINS = 100


def _conv3x3_same(x, w, b):
    # x: (Bl, C, H, W); w: (O, C, 3, 3). Implemented as 9 shifted adds so it
    # lowers to pad/slice/dot ops only (robust on the axon backend).
    Bl, C, H, W = x.shape
    O = w.shape[0]
    xp = jnp.pad(x, ((0, 0), (0, 0), (1, 1), (1, 1)))
    out = jnp.zeros((Bl, O, H, W), jnp.float32)
    for dy in range(3):
        for dx in range(3):
            patch = xp[:, :, dy:dy + H, dx:dx + W]  # (Bl, C, H, W)
            # contract channel dim: (O,C) x (Bl,C,H,W) -> (Bl,O,H,W)
            out = out + jnp.einsum('oc,bchw->bohw', w[:, :, dy, dx], patch)
    return out + b[None, :, None, None]


def _avgpool2(x):
    Bl, C, H, W = x.shape
    x = x.reshape(Bl, C, H // 2, 2, W // 2, 2)
    return x.mean(axis=(3, 5))


def _device_fn(idx_flat, valid_flat, z, y, p,
               c1w, c1b, c2w, c2b, c3w, c3b, f1w, f1b, f2w, f2b):
    # z, y, p: (Bl, 2, H, W) shards
    FQ_BOUND = 1.0
    TEMPERATURE = 0.1
    Bl = z.shape[0]

    radius_factor = jnp.arange(1, _NBINS + 1, dtype=jnp.float32) * 0.01

    abs_y = jnp.sqrt(y[:, 0] ** 2 + y[:, 1] ** 2)
    abs_z = jnp.sqrt(z[:, 0] ** 2 + z[:, 1] ** 2)
    fi = jnp.stack([abs_y, jnp.log10(abs_y + 1e-10),
                    abs_z, jnp.log10(abs_z + 1e-10)], axis=1)

    h = jax.nn.relu(_conv3x3_same(fi, c1w, c1b))
    h = _avgpool2(h)
    h = jax.nn.relu(_conv3x3_same(h, c2w, c2b))
    h = _avgpool2(h)
    h = jax.nn.relu(_conv3x3_same(h, c3w, c3b))
    feat = h.mean(axis=(2, 3))                      # (Bl, 64)

    v = feat @ f1w.T + f1b                          # (Bl, 256)
    logits = (v @ f2w.T + f2b).reshape(Bl, _NBINS, _NBINS)
    sm = jax.nn.softmax(logits * TEMPERATURE, axis=-1) * radius_factor[None, None, :]
    value_set = sm.sum(axis=-1) * FQ_BOUND          # (Bl, 100)

    # Static radial binning: idx_flat (H*W,) int32 in [0, 99] (pre-clipped),
    # valid_flat (H*W,) f32 in {0,1} for idx < 100.
    gathered = jnp.take(value_set, idx_flat, axis=1)          # (Bl, H*W)
    fq_mask = (gathered * valid_flat[None, :]).reshape(Bl, 1, _H, _W)

    m = fq_mask
    pr, pi = p[:, 0], p[:, 1]
    zr, zi = z[:, 0], z[:, 1]
    yr, yi = y[:, 0], y[:, 1]
    m0 = m[:, 0]

    # replaced_z = z + conj(p) * (y - z*m)
    dr = yr - zr * m0
    di = yi - zi * m0
    rz_r = zr + pr * dr + pi * di
    rz_i = zi + pr * di - pi * dr

    # replaced_y = y + (conj(p)/|p|^2) * (z - y*m)
    mag2 = jnp.maximum(pr ** 2 + pi ** 2, 1e-10)
    er = zr - yr * m0
    ei = zi - yi * m0
    sr = pr / mag2
    si = -pi / mag2
    ry_r = yr + sr * er - si * ei
    ry_i = yi + sr * ei + si * er

    replaced_z = jnp.stack([rz_r, rz_i], axis=1)
    replaced_y = jnp.stack([ry_r, ry_i], axis=1)
    return replaced_z, replaced_y, fq_mask


_PMAPPED = None


def _get_pmapped():
    global _PMAPPED
    if _PMAPPED is None:
        idx_clip, valid = _static_bins()
        fn = partial(_device_fn, jnp.asarray(idx_clip), jnp.asarray(valid))
        _PMAPPED = jax.pmap(
            fn,
            in_axes=(0, 0, 0) + (None,) * 10,
            devices=jax.devices()[:_NCORES],
        )
    return _PMAPPED


def _static_bins():
    # Mirror the reference's fp32 arithmetic exactly.
    radius_factor = (np.arange(1, _NBINS + 1, dtype=np.float32)) * np.float32(0.01)
    max_radius = math.sqrt(_H * _H + _W * _W) / 2.0
    ii = np.arange(_H, dtype=np.float32)
    jj = np.arange(_W, dtype=np.float32)
    dist = np.sqrt((ii[:, None] - np.float32(_H / 2.0)) ** 2
                   + (jj[None, :] - np.float32(_W / 2.0)) ** 2).astype(np.float32)
    radius_set = (np.float32(max_radius) * radius_factor).astype(np.float32)
    idx = np.searchsorted(radius_set, dist.ravel(), side='right').astype(np.int32)
    valid = (idx < _NBINS).astype(np.float32)
    idx_clip = np.clip(idx, 0, _NBINS - 1).astype(np.int32)
    return idx_clip, valid


def kernel(z_clean_fq, y_noise_fq, prbs_fq,
           conv1_w, conv1_b, conv2_w, conv2_b, conv3_w, conv3_b,
           fc1_w, fc1_b, fc2_w, fc2_b):
    f = _get_pmapped()

    def shard(a):
        a = np.asarray(a, np.float32)
        return a.reshape(_NCORES, _B // _NCORES, *a.shape[1:])

    rz, ry, fm = f(shard(z_clean_fq), shard(y_noise_fq), shard(prbs_fq),
                   jnp.asarray(conv1_w), jnp.asarray(conv1_b),
                   jnp.asarray(conv2_w), jnp.asarray(conv2_b),
                   jnp.asarray(conv3_w), jnp.asarray(conv3_b),
                   jnp.asarray(fc1_w), jnp.asarray(fc1_b),
                   jnp.asarray(fc2_w), jnp.asarray(fc2_b))

    rz = np.asarray(rz, np.float32).reshape(_B, 2, _H, _W)
    ry = np.asarray(ry, np.float32).reshape(_B, 2, _H, _W)
    fm = np.asarray(fm, np.float32).reshape(_B, 1, _H, _W)
    return rz, ry, fm


# revision 8
# speedup vs baseline: 7.9887x; 7.9887x over previous
import math
from functools import partial

import jax
import jax.numpy as jnp
import numpy as np

# nn_AFM_PRBS: data-parallel over batch across 8 NeuronCores.
# B=16, H=W=256, N_BINS=100. Each core gets B/8 = 2 batches; all params
# are replicated (tiny CNN + FC); dist/radius constants are recomputed
# per device (they are static functions of H, W).

_B, _H, _W = 16, 256, 256
_NCORES = 8
_NBINS = 100


def _conv3x3_same(x, w, b):
    # x: (Bl, C, H, W); w: (O, C, 3, 3). Implemented as 9 shifted adds so it
    # lowers to pad/slice/dot ops only (robust on the axon backend).
    Bl, C, H, W = x.shape
    O = w.shape[0]
    xp = jnp.pad(x, ((0, 0), (0, 0), (1, 1), (1, 1)))
    out = jnp.zeros((Bl, O, H, W), jnp.float32)
    for dy in range(3):
        for dx in range(3):
            patch = xp[:, :, dy:dy + H, dx:dx + W]  # (Bl, C, H, W)
            # contract channel dim: (O,C) x (Bl,C,H,W) -> (Bl,O,H,W)
            out = out + jnp.einsum('oc,bchw->bohw', w[:, :, dy, dx], patch)
    return out + b[None, :, None, None]


def _avgpool2(x):
    Bl, C, H, W = x.shape
    x = x.reshape(Bl, C, H // 2, 2, W // 2, 2)
    return x.mean(axis=(3, 5))


def _device_fn(idx_flat, valid_flat, z, y, p,
               c1w, c1b, c2w, c2b, c3w, c3b, f1w, f1b, f2w, f2b):
    # z, y, p: (Bl, 2, H, W) shards
    FQ_BOUND = 1.0
    TEMPERATURE = 0.1
    Bl = z.shape[0]

    radius_factor = jnp.arange(1, _NBINS + 1, dtype=jnp.float32) * 0.01

    abs_y = jnp.sqrt(y[:, 0] ** 2 + y[:, 1] ** 2)
    abs_z = jnp.sqrt(z[:, 0] ** 2 + z[:, 1] ** 2)
    fi = jnp.stack([abs_y, jnp.log10(abs_y + 1e-10),
                    abs_z, jnp.log10(abs_z + 1e-10)], axis=1)

    h = jax.nn.relu(_conv3x3_same(fi, c1w, c1b))
    h = _avgpool2(h)
    h = jax.nn.relu(_conv3x3_same(h, c2w, c2b))
    h = _avgpool2(h)
    h = jax.nn.relu(_conv3x3_same(h, c3w, c3b))
    feat = h.mean(axis=(2, 3))                      # (Bl, 64)

    v = feat @ f1w.T + f1b                          # (Bl, 256)
    logits = (v @ f2w.T + f2b).reshape(Bl, _NBINS, _NBINS)
    sm = jax.nn.softmax(logits * TEMPERATURE, axis=-1) * radius_factor[None, None, :]
    value_set = sm.sum(axis=-1) * FQ_BOUND          # (Bl, 100)

    # Static radial binning: idx_flat (H*W,) int32 in [0, 99] (pre-clipped),
    # valid_flat (H*W,) f32 in {0,1} for idx < 100.
    gathered = jnp.take(value_set, idx_flat, axis=1)          # (Bl, H*W)
    fq_mask = (gathered * valid_flat[None, :]).reshape(Bl, 1, _H, _W)

    m = fq_mask
    pr, pi = p[:, 0], p[:, 1]
    zr, zi = z[:, 0], z[:, 1]
    yr, yi = y[:, 0], y[:, 1]
    m0 = m[:, 0]

    # replaced_z = z + conj(p) * (y - z*m)
    dr = yr - zr * m0
    di = yi - zi * m0
    rz_r = zr + pr * dr + pi * di
    rz_i = zi + pr * di - pi * dr

    # replaced_y = y + (conj(p)/|p|^2) * (z - y*m)
    mag2 = jnp.maximum(pr ** 2 + pi ** 2, 1e-10)
    er = zr - yr * m0
    ei = zi - yi * m0
    sr = pr / mag2
    si = -pi / mag2
    ry_r = yr + sr * er - si * ei
    ry_i = yi + sr * ei + si * er

    replaced_z = jnp.stack([rz_r, rz_i], axis=1)
    replaced_y = jnp.stack([ry_r, ry_i], axis=1)
    return replaced_z, replaced_y, fq_mask


_PMAPPED = None
_NUSED = _NCORES


def _get_pmapped():
    global _PMAPPED, _NUSED
    if _PMAPPED is None:
        _NUSED = min(_NCORES, len(jax.devices()), _B)
        while _B % _NUSED:
            _NUSED -= 1
        idx_clip, valid = _static_bins()
        fn = partial(_device_fn, jnp.asarray(idx_clip), jnp.asarray(valid))
        _PMAPPED = jax.pmap(
            fn,
            in_axes=(0, 0, 0) + (None,) * 10,
            devices=jax.devices()[:_NUSED],
        )
    return _PMAPPED


def _static_bins():
    # Mirror the reference's fp32 arithmetic exactly.
    radius_factor = (np.arange(1, _NBINS + 1, dtype=np.float32)) * np.float32(0.01)
    max_radius = math.sqrt(_H * _H + _W * _W) / 2.0
    ii = np.arange(_H, dtype=np.float32)
    jj = np.arange(_W, dtype=np.float32)
    dist = np.sqrt((ii[:, None] - np.float32(_H / 2.0)) ** 2
                   + (jj[None, :] - np.float32(_W / 2.0)) ** 2).astype(np.float32)
    radius_set = (np.float32(max_radius) * radius_factor).astype(np.float32)
    idx = np.searchsorted(radius_set, dist.ravel(), side='right').astype(np.int32)
    valid = (idx < _NBINS).astype(np.float32)
    idx_clip = np.clip(idx, 0, _NBINS - 1).astype(np.int32)
    return idx_clip, valid


def kernel(z_clean_fq, y_noise_fq, prbs_fq,
           conv1_w, conv1_b, conv2_w, conv2_b, conv3_w, conv3_b,
           fc1_w, fc1_b, fc2_w, fc2_b):
    f = _get_pmapped()

    def shard(a):
        a = np.asarray(a, np.float32)
        return a.reshape(_NUSED, _B // _NUSED, *a.shape[1:])

    rz, ry, fm = f(shard(z_clean_fq), shard(y_noise_fq), shard(prbs_fq),
                   jnp.asarray(conv1_w), jnp.asarray(conv1_b),
                   jnp.asarray(conv2_w), jnp.asarray(conv2_b),
                   jnp.asarray(conv3_w), jnp.asarray(conv3_b),
                   jnp.asarray(fc1_w), jnp.asarray(fc1_b),
                   jnp.asarray(fc2_w), jnp.asarray(fc2_b))

    rz = np.asarray(rz, np.float32).reshape(_B, 2, _H, _W)
    ry = np.asarray(ry, np.float32).reshape(_B, 2, _H, _W)
    fm = np.asarray(fm, np.float32).reshape(_B, 1, _H, _W)
    return rz, ry, fm


# revision 9
# speedup vs baseline: 8.3865x; 1.0498x over previous
import math
from functools import partial

import jax
import jax.numpy as jnp
import numpy as np

# nn_AFM_PRBS: data-parallel over batch across 8 NeuronCores.
# B=16, H=W=256, N_BINS=100. Each core gets B/8 = 2 batches; all params
# are replicated (tiny CNN + FC); dist/radius constants are recomputed
# per device (they are static functions of H, W).

_B, _H, _W = 16, 256, 256
_NCORES = 8
_NBINS = 100


def _conv3x3_same(x, w, b):
    # x: (Bl, C, H, W); w: (O, C, 3, 3). Implemented as 9 shifted adds so it
    # lowers to pad/slice/dot ops only (robust on the axon backend).
    Bl, C, H, W = x.shape
    O = w.shape[0]
    xp = jnp.pad(x, ((0, 0), (0, 0), (1, 1), (1, 1))).astype(jnp.bfloat16)
    wb = w.astype(jnp.bfloat16)
    out = jnp.zeros((Bl, O, H, W), jnp.float32)
    for dy in range(3):
        for dx in range(3):
            patch = xp[:, :, dy:dy + H, dx:dx + W]  # (Bl, C, H, W)
            # contract channel dim: (O,C) x (Bl,C,H,W) -> (Bl,O,H,W)
            out = out + jnp.einsum('oc,bchw->bohw', wb[:, :, dy, dx], patch,
                                   preferred_element_type=jnp.float32)
    return out + b[None, :, None, None]


def _avgpool2(x):
    Bl, C, H, W = x.shape
    x = x.reshape(Bl, C, H // 2, 2, W // 2, 2)
    return x.mean(axis=(3, 5))


def _device_fn(idx_flat, valid_flat, z, y, p,
               c1w, c1b, c2w, c2b, c3w, c3b, f1w, f1b, f2w, f2b):
    # z, y, p: (Bl, 2, H, W) shards
    FQ_BOUND = 1.0
    TEMPERATURE = 0.1
    Bl = z.shape[0]

    radius_factor = jnp.arange(1, _NBINS + 1, dtype=jnp.float32) * 0.01

    abs_y = jnp.sqrt(y[:, 0] ** 2 + y[:, 1] ** 2)
    abs_z = jnp.sqrt(z[:, 0] ** 2 + z[:, 1] ** 2)
    fi = jnp.stack([abs_y, jnp.log10(abs_y + 1e-10),
                    abs_z, jnp.log10(abs_z + 1e-10)], axis=1)

    h = jax.nn.relu(_conv3x3_same(fi, c1w, c1b))
    h = _avgpool2(h)
    h = jax.nn.relu(_conv3x3_same(h, c2w, c2b))
    h = _avgpool2(h)
    h = jax.nn.relu(_conv3x3_same(h, c3w, c3b))
    feat = h.mean(axis=(2, 3))                      # (Bl, 64)

    v = feat @ f1w.T + f1b                          # (Bl, 256)
    logits = (v @ f2w.T + f2b).reshape(Bl, _NBINS, _NBINS)
    sm = jax.nn.softmax(logits * TEMPERATURE, axis=-1) * radius_factor[None, None, :]
    value_set = sm.sum(axis=-1) * FQ_BOUND          # (Bl, 100)

    # Static radial binning: idx_flat (H*W,) int32 in [0, 99] (pre-clipped),
    # valid_flat (H*W,) f32 in {0,1} for idx < 100.
    gathered = jnp.take(value_set, idx_flat, axis=1)          # (Bl, H*W)
    fq_mask = (gathered * valid_flat[None, :]).reshape(Bl, 1, _H, _W)

    m = fq_mask
    pr, pi = p[:, 0], p[:, 1]
    zr, zi = z[:, 0], z[:, 1]
    yr, yi = y[:, 0], y[:, 1]
    m0 = m[:, 0]

    # replaced_z = z + conj(p) * (y - z*m)
    dr = yr - zr * m0
    di = yi - zi * m0
    rz_r = zr + pr * dr + pi * di
    rz_i = zi + pr * di - pi * dr

    # replaced_y = y + (conj(p)/|p|^2) * (z - y*m)
    mag2 = jnp.maximum(pr ** 2 + pi ** 2, 1e-10)
    er = zr - yr * m0
    ei = zi - yi * m0
    sr = pr / mag2
    si = -pi / mag2
    ry_r = yr + sr * er - si * ei
    ry_i = yi + sr * ei + si * er

    replaced_z = jnp.stack([rz_r, rz_i], axis=1)
    replaced_y = jnp.stack([ry_r, ry_i], axis=1)
    return replaced_z, replaced_y, fq_mask


_PMAPPED = None
_NUSED = _NCORES


def _get_pmapped():
    global _PMAPPED, _NUSED
    if _PMAPPED is None:
        _NUSED = min(_NCORES, len(jax.devices()), _B)
        while _B % _NUSED:
            _NUSED -= 1
        idx_clip, valid = _static_bins()
        fn = partial(_device_fn, jnp.asarray(idx_clip), jnp.asarray(valid))
        _PMAPPED = jax.pmap(
            fn,
            in_axes=(0, 0, 0) + (None,) * 10,
            devices=jax.devices()[:_NUSED],
        )
    return _PMAPPED


def _static_bins():
    # Mirror the reference's fp32 arithmetic exactly.
    radius_factor = (np.arange(1, _NBINS + 1, dtype=np.float32)) * np.float32(0.01)
    max_radius = math.sqrt(_H * _H + _W * _W) / 2.0
    ii = np.arange(_H, dtype=np.float32)
    jj = np.arange(_W, dtype=np.float32)
    dist = np.sqrt((ii[:, None] - np.float32(_H / 2.0)) ** 2
                   + (jj[None, :] - np.float32(_W / 2.0)) ** 2).astype(np.float32)
    radius_set = (np.float32(max_radius) * radius_factor).astype(np.float32)
    idx = np.searchsorted(radius_set, dist.ravel(), side='right').astype(np.int32)
    valid = (idx < _NBINS).astype(np.float32)
    idx_clip = np.clip(idx, 0, _NBINS - 1).astype(np.int32)
    return idx_clip, valid


def kernel(z_clean_fq, y_noise_fq, prbs_fq,
           conv1_w, conv1_b, conv2_w, conv2_b, conv3_w, conv3_b,
           fc1_w, fc1_b, fc2_w, fc2_b):
    f = _get_pmapped()

    def shard(a):
        a = np.asarray(a, np.float32)
        return a.reshape(_NUSED, _B // _NUSED, *a.shape[1:])

    rz, ry, fm = f(shard(z_clean_fq), shard(y_noise_fq), shard(prbs_fq),
                   jnp.asarray(conv1_w), jnp.asarray(conv1_b),
                   jnp.asarray(conv2_w), jnp.asarray(conv2_b),
                   jnp.asarray(conv3_w), jnp.asarray(conv3_b),
                   jnp.asarray(fc1_w), jnp.asarray(fc1_b),
                   jnp.asarray(fc2_w), jnp.asarray(fc2_b))

    rz = np.asarray(rz, np.float32).reshape(_B, 2, _H, _W)
    ry = np.asarray(ry, np.float32).reshape(_B, 2, _H, _W)
    fm = np.asarray(fm, np.float32).reshape(_B, 1, _H, _W)
    return rz, ry, fm
